# revision 7
# baseline (speedup 1.0000x reference)
"""Trainium2 Bass kernel for nn_AttentionLayer (pooling, dim=0 softmax).

Computation (full shapes B=64, T=2048, D=256):
    u = tanh(hs @ W^T + b)            [B,T,D]
    scores = u @ v                    [B,T]
    a = softmax(scores, axis=0)       (over the batch axis!)
    s[b] = a[b] @ hs[b]               [B,D]

Sharding: sequence-parallel over T across 8 cores (T_loc = 256). The
dim=0 softmax couples samples but not time steps, so each core's
softmax is fully local; only the final weighted sum needs a cross-core
reduction, done on the host (8 x 64KB partials).

Per-core row ordering is (c, b, t_lo) with c = t//128: the softmax for
the c=0 half of the time steps completes at the loop midpoint, so its
weighted-sum matmuls interleave with the second half's GEMM instead of
serializing at the end.

Per-core device pipeline (fp16 compute, f32 PSUM accumulation):
  1. plain DMA loads of xt (d-major) and xn (natural) fp16 group tiles
  2. PE mm1: z^T[e, r] = W-chunk @ Xt-chunk  (PSUM f32)
  3. ACT: u = tanh(z + bias)  PSUM -> SBUF fp16, per-partition bias
  4. DVE: uv = u0*v0 + u1*v1 (tensor_scalar + scalar_tensor_tensor),
     collapsing the e-contraction's elementwise part off the PE
  5. PE: ones^T @ uv -> scores row [1, 512] (partition reduction)
  6. DVE/GPSIMD copy scores PSUM->SBUF row; SBUF->SBUF scatter DMA
     lands rows b of each group in a per-half stage tile [64, 128]
  7. per half: PE transpose stage -> scmat [128 t_lo, 64 b] f32,
     softmax over b (free dim), normalized weights written into a
     block-diagonal slab via a stride-65 access pattern
  8. PE step4: 64 matmuls per half, lhsT=slab[:, b, :], rhs=xn[c,b],
     all 128 accumulate one [64, 256] PSUM tile; c=0 half issued
     between the second half's mm1 groups
  9. one PSUM->SBUF copy + DMA out s_partial [64, 256] f32; host sums
     the 8 partials.
"""

import numpy as np

B, T, D = 64, 2048, 256
NCORES = 8
T_LOC = T // NCORES          # 256
BT = B * T_LOC               # 16384 rows per core
BTG = 1024                   # rows per pipeline group (8 b x 128 t_lo)
PH = 128                     # partitions
NG = BT // BTG               # 16 groups (8 per c-half)
GPH = NG // 2                # groups per half


def build_program():
    import concourse.bacc as bacc
    import concourse.tile as tile
    from concourse import mybir

    F32 = mybir.dt.float32
    F16 = mybir.dt.float16
    AF = mybir.ActivationFunctionType
    AX = mybir.AxisListType
    MUL = mybir.AluOpType.mult
    ADD = mybir.AluOpType.add

    nc = bacc.Bacc("TRN2", target_bir_lowering=False, debug=False)

    # Host-prepacked fp16 inputs (see prep_core_inputs below). Row order
    # is r = c*8192 + b*128 + t_lo  (c = t//128, t_lo = t%128).
    #   xt16[g, p, m, q] = X^T tiles: g = c*8+gl, m = 2*i+dc covers rows
    #       (gl*8+i)*128+q of half c, d = dc*128+p
    #   xn16[c, gl, p, i, d] = X[row (c, gl*8+i, t_lo=p), d]
    xt_d = nc.dram_tensor("xt16", [NG, PH, 16, PH], F16, kind="ExternalInput").ap()
    xn_d = nc.dram_tensor(
        "xn16", [2, GPH, PH, 8, D], F16, kind="ExternalInput"
    ).ap()
    wt_d = nc.dram_tensor("wt16", [PH, 2, 2, PH], F16, kind="ExternalInput").ap()
    bias_d = nc.dram_tensor("bias2", [PH, 2], F32, kind="ExternalInput").ap()
    v_d = nc.dram_tensor("v2", [PH, 2], F32, kind="ExternalInput").ap()
    out = nc.dram_tensor("out", [B, D], F32, kind="ExternalOutput").ap()

    with tile.TileContext(nc) as tc:
        with (
            tc.tile_pool(name="singles", bufs=1) as singles,
            tc.tile_pool(name="xnat", bufs=NG) as xnat_pool,
            tc.tile_pool(name="xt", bufs=4) as xt_pool,
            tc.tile_pool(name="usb", bufs=4) as u_pool,
            tc.tile_pool(name="uv", bufs=3) as uv_pool,
            tc.tile_pool(name="scrow", bufs=3) as scrow_pool,
            tc.tile_pool(name="small", bufs=8) as small,
            tc.tile_pool(name="dram", bufs=3, space="DRAM") as dram_pool,
        ):
            # ---- constants (wt first: it gates the first matmul) ----
            wt = singles.tile([PH, 2, 2, PH], F16)
            nc.sync.dma_start(out=wt, in_=wt_d)
            bias_sb = singles.tile([PH, 2], F32)
            nc.gpsimd.dma_start(out=bias_sb, in_=bias_d)
            v32 = singles.tile([PH, 2], F32)
            nc.gpsimd.dma_start(out=v32, in_=v_d)
            ones16 = singles.tile([PH, 1], F16)
            nc.vector.memset(ones16, 1.0)
            identity32 = singles.tile([PH, PH], F32)
            from concourse.masks import make_identity
            make_identity(nc, identity32)

            # per-half staging of score rows: stage[c][b, t_lo]
            stages = [
                singles.tile([B, PH], F32, name=f"stage{c}", tag=f"stage{c}")
                for c in (0, 1)
            ]
            scmats = [
                singles.tile([PH, B], F32, name=f"scmat{c}", tag=f"scmat{c}")
                for c in (0, 1)
            ]
            # block-diagonal weight slabs, one per half
            slabs = [
                singles.tile([PH, B, B], F16, name=f"slab{c}", tag=f"slab{c}")
                for c in (0, 1)
            ]
            for c in (0, 1):
                nc.vector.memset(slabs[c], 0.0)
            s_sb = singles.tile([B, D], F32)

            xnat_tiles = []

            with (
                tc.tile_pool(name="ups", bufs=2, space="PSUM") as ups_pool,
                tc.tile_pool(name="scps", bufs=2, space="PSUM") as scps_pool,
                tc.tile_pool(name="s4ps", bufs=1, space="PSUM") as s4_pool,
                tc.tile_pool(name="tps", bufs=1, space="PSUM") as t_pool,
            ):
                s_ps = s4_pool.tile([B, D], F32)
                step4_done = 0  # chunks issued so far (0..128)

                def softmax_half(c):
                    # stage[c] rows complete -> transpose -> softmax ->
                    # slab diagonal write (stride B+1)
                    t_ps = t_pool.tile([PH, B], F32)
                    nc.tensor.transpose(
                        t_ps, stages[c], identity32[0:B, 0:B]
                    )
                    nc.vector.tensor_copy(scmats[c], t_ps)
                    nm = small.tile([PH, 1], F32)
                    nc.vector.reduce_max(
                        nm, scmats[c], axis=AX.X, negate=True
                    )
                    e_sb = small.tile([PH, B], F32)
                    ssum = small.tile([PH, 1], F32)
                    nc.scalar.activation(
                        e_sb, scmats[c], AF.Exp, bias=nm, accum_out=ssum
                    )
                    rec = small.tile([PH, 1], F32)
                    nc.vector.reciprocal(rec, ssum)
                    slab_flat = slabs[c].rearrange("p j b -> p (j b)")
                    st = B + 1
                    diag = slab_flat[:, 0:(B - 1) * st + 1:st]
                    nc.vector.tensor_scalar_mul(diag, e_sb, rec)

                def issue_step4(n):
                    # issue n more weighted-sum chunks (in (c, b) order)
                    nonlocal step4_done
                    for k in range(step4_done, step4_done + n):
                        c, b = k // B, k % B
                        nc.tensor.matmul(
                            s_ps,
                            slabs[c][:, b, :],
                            xnat_tiles[c * GPH + b // 8][:, b % 8, :],
                            start=(k == 0),
                            stop=(k == 2 * B - 1),
                        )
                    step4_done += n

                for g in range(NG):
                    c, gl = g // GPH, g % GPH
                    # ---- loads: xt alternates sync/scalar HWDGE queues,
                    #      xn on the gpsimd SWDGE queue ----
                    xt = xt_pool.tile([PH, 16, PH], F16)
                    eng = nc.scalar if g % 2 == 0 else nc.sync
                    eng.dma_start(out=xt, in_=xt_d[g])
                    xn = xnat_pool.tile([PH, 8, D], F16)
                    nc.gpsimd.dma_start(out=xn, in_=xn_d[c, gl])
                    xnat_tiles.append(xn)

                    # ---- mm1 + tanh ----
                    u16 = []
                    for ec in range(2):
                        u_ps = ups_pool.tile([PH, BTG], F32)
                        for half in range(2):
                            for dc in range(2):
                                m0 = half * 8 + dc
                                nc.tensor.matmul(
                                    u_ps[:, half * 512:(half + 1) * 512],
                                    wt[:, dc, ec, :],
                                    xt[:, m0:m0 + 7:2, :],
                                    start=(dc == 0),
                                    stop=(dc == 1),
                                )
                        u_sb = u_pool.tile([PH, BTG], F16)
                        nc.scalar.activation(
                            u_sb, u_ps, AF.Tanh, bias=bias_sb[:, ec:ec + 1]
                        )
                        u16.append(u_sb)

                    # ---- uv = u0*v0 + u1*v1 on DVE ----
                    t0 = uv_pool.tile([PH, BTG], F16, tag="t0")
                    nc.vector.tensor_scalar_mul(t0, u16[0], v32[:, 0:1])
                    uv = uv_pool.tile([PH, BTG], F16, tag="uv")
                    nc.vector.scalar_tensor_tensor(
                        uv, u16[1], v32[:, 1:2], t0, MUL, ADD
                    )

                    # ---- scores row: ones^T @ uv, copy PSUM->SBUF, then
                    # bounce through DRAM to scatter rows b across the
                    # stage tile's partitions ----
                    scrow = scrow_pool.tile([1, BTG], F32)
                    for half in range(2):
                        sc_ps = scps_pool.tile([1, 512], F32)
                        nc.tensor.matmul(
                            sc_ps,
                            ones16,
                            uv[:, half * 512:(half + 1) * 512],
                            start=True,
                            stop=True,
                        )
                        if half == 0:
                            nc.vector.tensor_copy(
                                scrow[0:1, 0:512], sc_ps
                            )
                        else:
                            nc.scalar.copy(scrow[0:1, 512:1024], sc_ps)
                    scd = dram_pool.tile([1, BTG], F32, tag="scd")
                    nc.sync.dma_start(out=scd, in_=scrow)
                    nc.sync.dma_start(
                        out=stages[c][gl * 8:gl * 8 + 8, :],
                        in_=scd.rearrange("a (r q) -> (a r) q", q=PH),
                    )

                    # ---- half-boundary work: softmax for c=0 is issued
                    # after group 8's mm1 so the PE transpose never waits
                    # on the stage scatter DMAs ----
                    if g == GPH:
                        softmax_half(0)
                    if g >= GPH + 1:
                        # spread the 64 c=0 chunks over groups 9..15
                        issue_step4(9 if g > GPH + 1 else 10)

                softmax_half(1)
                issue_step4(2 * B - step4_done)

                nc.vector.tensor_copy(s_sb, s_ps)
                nc.sync.dma_start(out=out, in_=s_sb)

    nc.compile()
    return nc


_prog_cache = {}


def _get_program(b_dim=B):
    if "p" not in _prog_cache:
        _prog_cache["p"] = build_program()
    return _prog_cache["p"]


def prep_core_inputs(shard_f32, w, bias, v):
    """Pack one core's [B, T_LOC, D] f32 shard + weights into device
    layouts. Row order: r = c*8192 + b*128 + t_lo."""
    h16 = shard_f32.astype(np.float16)          # [64, 256, 256]
    # [b, c, t_lo, d]
    hr = h16.reshape(B, 2, PH, D)
    # xn16[c, gl, p=t_lo, i=b%8, d]
    xn16 = np.ascontiguousarray(
        hr.reshape(8, 8, 2, PH, D).transpose(2, 0, 3, 1, 4)
    ).reshape(2, GPH, PH, 8, D)
    # xt16[g, p, m, q] = X[row=(c, gl*8+i, q), dc*128+p]
    #   hr -> [b, c, q, dc, p] -> [c, gl, i, q, dc, p] -> [c, gl, p, i, dc, q]
    hx = hr.reshape(8, 8, 2, PH, 2, PH).transpose(2, 0, 5, 1, 4, 3)
    xt16 = np.ascontiguousarray(hx).reshape(NG, PH, 16, PH)
    # wt16[p, dc, ec, e'] = W[ec*128 + e', dc*128 + p]
    w16 = w.astype(np.float16)
    wt16 = np.ascontiguousarray(
        w16.reshape(2, PH, 2, PH).transpose(3, 2, 0, 1)
    )
    bias2 = np.ascontiguousarray(bias.reshape(2, PH).T).astype(np.float32)
    v2 = np.ascontiguousarray(v.reshape(2, PH).T).astype(np.float32)
    return {
        "xn16": xn16,
        "xt16": xt16,
        "wt16": wt16,
        "bias2": bias2,
        "v2": v2,
    }


def kernel(hidden_states, W_attention, bias_attention, attention_vector):
    from concourse.bass_utils import run_bass_kernel_spmd

    hs = np.asarray(hidden_states, dtype=np.float32)
    w = np.asarray(W_attention, dtype=np.float32)
    bias = np.asarray(bias_attention, dtype=np.float32)
    v = np.asarray(attention_vector, dtype=np.float32)

    nc = _get_program()

    in_maps = []
    for core in range(NCORES):
        shard = np.ascontiguousarray(
            hs[:, core * T_LOC:(core + 1) * T_LOC, :]
        )
        in_maps.append(prep_core_inputs(shard, w, bias, v))

    res = run_bass_kernel_spmd(nc, in_maps, list(range(NCORES)))
    s = np.zeros((B, D), dtype=np.float32)
    for r in res.results:
        s += r["out"]
    return s


# revision 9
# speedup vs baseline: 1.0378x; 1.0378x over previous
"""Trainium2 Bass kernel for nn_AttentionLayer (pooling, dim=0 softmax).

Computation (full shapes B=64, T=2048, D=256):
    u = tanh(hs @ W^T + b)            [B,T,D]
    scores = u @ v                    [B,T]
    a = softmax(scores, axis=0)       (over the batch axis!)
    s[b] = a[b] @ hs[b]               [B,D]

Sharding: sequence-parallel over T across 8 cores (T_loc = 256). The
dim=0 softmax couples samples but not time steps, so each core's
softmax is fully local; only the final weighted sum needs a cross-core
reduction, done on the host (8 x 64KB partials).

Per-core row ordering is (c, b, t_lo) with c = t//128: the softmax for
the c=0 half of the time steps completes at the loop midpoint, so its
weighted-sum matmuls interleave with the second half's GEMM instead of
serializing at the end.

Per-core device pipeline (fp16 compute, f32 PSUM accumulation):
  1. plain DMA loads of xt (d-major) and xn (natural) fp16 group tiles
  2. PE mm1: z^T[e, r] = W-chunk @ Xt-chunk  (PSUM f32)
  3. ACT: u = tanh(z + bias)  PSUM -> SBUF fp16, per-partition bias
  4. DVE: uv = u0*v0 + u1*v1 (tensor_scalar + scalar_tensor_tensor),
     collapsing the e-contraction's elementwise part off the PE
  5. PE: ones^T @ uv -> scores row [1, 512] (partition reduction)
  6. DVE/GPSIMD copy scores PSUM->SBUF row; SBUF->SBUF scatter DMA
     lands rows b of each group in a per-half stage tile [64, 128]
  7. per half: PE transpose stage -> scmat [128 t_lo, 64 b] f32,
     softmax over b (free dim), normalized weights written into a
     block-diagonal slab via a stride-65 access pattern
  8. PE step4: 64 matmuls per half, lhsT=slab[:, b, :], rhs=xn[c,b],
     all 128 accumulate one [64, 256] PSUM tile; c=0 half issued
     between the second half's mm1 groups
  9. one PSUM->SBUF copy + DMA out s_partial [64, 256] f32; host sums
     the 8 partials.
"""

import numpy as np

B, T, D = 64, 2048, 256
NCORES = 8
T_LOC = T // NCORES          # 256
BT = B * T_LOC               # 16384 rows per core
BTG = 1024                   # rows per pipeline group (8 b x 128 t_lo)
PH = 128                     # partitions
NG = BT // BTG               # 16 groups (8 per c-half)
GPH = NG // 2                # groups per half


def build_program():
    import concourse.bacc as bacc
    import concourse.tile as tile
    from concourse import mybir

    F32 = mybir.dt.float32
    F16 = mybir.dt.float16
    AF = mybir.ActivationFunctionType
    AX = mybir.AxisListType
    MUL = mybir.AluOpType.mult
    ADD = mybir.AluOpType.add

    nc = bacc.Bacc("TRN2", target_bir_lowering=False, debug=False)

    # Host-prepacked fp16 inputs (see prep_core_inputs below). Row order
    # is r = c*8192 + b*128 + t_lo  (c = t//128, t_lo = t%128).
    #   xt16[g, p, m, q] = X^T tiles: g = c*8+gl, m = 2*i+dc covers rows
    #       (gl*8+i)*128+q of half c, d = dc*128+p
    #   xn16[c, gl, p, i, d] = X[row (c, gl*8+i, t_lo=p), d]
    xt_d = nc.dram_tensor("xt16", [NG, PH, 16, PH], F16, kind="ExternalInput").ap()
    xn_d = nc.dram_tensor(
        "xn16", [2, GPH, PH, 8, D], F16, kind="ExternalInput"
    ).ap()
    wt_d = nc.dram_tensor("wt16", [PH, 2, 2, PH], F16, kind="ExternalInput").ap()
    bias_d = nc.dram_tensor("bias2", [PH, 2], F32, kind="ExternalInput").ap()
    v_d = nc.dram_tensor("v2", [PH, 2], F32, kind="ExternalInput").ap()
    out = nc.dram_tensor("out", [B, D], F32, kind="ExternalOutput").ap()

    with tile.TileContext(nc) as tc:
        with (
            tc.tile_pool(name="singles", bufs=1) as singles,
            tc.tile_pool(name="xnat", bufs=NG) as xnat_pool,
            tc.tile_pool(name="xt", bufs=4) as xt_pool,
            tc.tile_pool(name="usb", bufs=4) as u_pool,
            tc.tile_pool(name="uv", bufs=3) as uv_pool,
            tc.tile_pool(name="scrow", bufs=3) as scrow_pool,
            tc.tile_pool(name="small", bufs=8) as small,
            tc.tile_pool(name="dram", bufs=3, space="DRAM") as dram_pool,
        ):
            # ---- constants (wt first: it gates the first matmul) ----
            wt = singles.tile([PH, 2, 2, PH], F16)
            nc.sync.dma_start(out=wt, in_=wt_d)
            bias_sb = singles.tile([PH, 2], F32)
            nc.gpsimd.dma_start(out=bias_sb, in_=bias_d)
            v32 = singles.tile([PH, 2], F32)
            nc.gpsimd.dma_start(out=v32, in_=v_d)
            ones16 = singles.tile([PH, 1], F16)
            nc.vector.memset(ones16, 1.0)
            identity32 = singles.tile([PH, PH], F32)
            from concourse.masks import make_identity
            make_identity(nc, identity32)

            # per-half staging of score rows: stage[c][b, t_lo]
            stages = [
                singles.tile([B, PH], F32, name=f"stage{c}", tag=f"stage{c}")
                for c in (0, 1)
            ]
            scmats = [
                singles.tile([PH, B], F32, name=f"scmat{c}", tag=f"scmat{c}")
                for c in (0, 1)
            ]
            # block-diagonal weight slabs, one per half
            slabs = [
                singles.tile([PH, B, B], F16, name=f"slab{c}", tag=f"slab{c}")
                for c in (0, 1)
            ]
            for c in (0, 1):
                nc.vector.memset(slabs[c], 0.0)
            s_sb = singles.tile([B, D], F32)

            xnat_tiles = []

            with (
                tc.tile_pool(name="ups", bufs=2, space="PSUM") as ups_pool,
                tc.tile_pool(name="scps", bufs=2, space="PSUM") as scps_pool,
                tc.tile_pool(name="s4ps", bufs=1, space="PSUM") as s4_pool,
                tc.tile_pool(name="tps", bufs=1, space="PSUM") as t_pool,
            ):
                s_ps = s4_pool.tile([B, D], F32)
                step4_done = 0  # chunks issued so far (0..128)

                def softmax_half(c):
                    # stage[c] rows complete -> transpose -> softmax ->
                    # slab diagonal write (stride B+1)
                    t_ps = t_pool.tile([PH, B], F32)
                    nc.tensor.transpose(
                        t_ps, stages[c], identity32[0:B, 0:B]
                    )
                    nc.vector.tensor_copy(scmats[c], t_ps)
                    nm = small.tile([PH, 1], F32)
                    nc.vector.reduce_max(
                        nm, scmats[c], axis=AX.X, negate=True
                    )
                    e_sb = small.tile([PH, B], F32)
                    ssum = small.tile([PH, 1], F32)
                    nc.scalar.activation(
                        e_sb, scmats[c], AF.Exp, bias=nm, accum_out=ssum
                    )
                    rec = small.tile([PH, 1], F32)
                    nc.vector.reciprocal(rec, ssum)
                    slab_flat = slabs[c].rearrange("p j b -> p (j b)")
                    st = B + 1
                    diag = slab_flat[:, 0:(B - 1) * st + 1:st]
                    nc.vector.tensor_scalar_mul(diag, e_sb, rec)

                def issue_step4(n):
                    # issue n more weighted-sum chunks (in (c, b) order)
                    nonlocal step4_done
                    for k in range(step4_done, step4_done + n):
                        c, b = k // B, k % B
                        nc.tensor.matmul(
                            s_ps,
                            slabs[c][:, b, :],
                            xnat_tiles[c * GPH + b // 8][:, b % 8, :],
                            start=(k == 0),
                            stop=(k == 2 * B - 1),
                        )
                    step4_done += n

                for g in range(NG):
                    c, gl = g // GPH, g % GPH
                    # ---- loads: xt alternates sync/scalar HWDGE queues,
                    #      xn on the gpsimd SWDGE queue ----
                    xt = xt_pool.tile([PH, 16, PH], F16)
                    eng = nc.scalar if g % 2 == 0 else nc.sync
                    eng.dma_start(out=xt, in_=xt_d[g])
                    xn = xnat_pool.tile([PH, 8, D], F16)
                    nc.gpsimd.dma_start(out=xn, in_=xn_d[c, gl])
                    xnat_tiles.append(xn)

                    # ---- mm1 + tanh ----
                    u16 = []
                    for ec in range(2):
                        u_ps = ups_pool.tile([PH, BTG], F32)
                        for half in range(2):
                            for dc in range(2):
                                m0 = half * 8 + dc
                                nc.tensor.matmul(
                                    u_ps[:, half * 512:(half + 1) * 512],
                                    wt[:, dc, ec, :],
                                    xt[:, m0:m0 + 7:2, :],
                                    start=(dc == 0),
                                    stop=(dc == 1),
                                )
                        u_sb = u_pool.tile([PH, BTG], F16)
                        nc.scalar.activation(
                            u_sb, u_ps, AF.Tanh, bias=bias_sb[:, ec:ec + 1]
                        )
                        u16.append(u_sb)

                    # ---- uv = u0*v0 + u1*v1 on DVE ----
                    t0 = uv_pool.tile([PH, BTG], F16, tag="t0")
                    nc.vector.tensor_scalar_mul(t0, u16[0], v32[:, 0:1])
                    t1 = uv_pool.tile([PH, BTG], F16, tag="t1")
                    nc.vector.tensor_scalar_mul(t1, u16[1], v32[:, 1:2])
                    uv = uv_pool.tile([PH, BTG], F16, tag="uv")
                    nc.vector.tensor_add(uv, t0, t1)

                    # ---- scores row: ones^T @ uv, copy PSUM->SBUF, then
                    # bounce through DRAM to scatter rows b across the
                    # stage tile's partitions ----
                    scrow = scrow_pool.tile([1, BTG], F32)
                    for half in range(2):
                        sc_ps = scps_pool.tile([1, 512], F32)
                        nc.tensor.matmul(
                            sc_ps,
                            ones16,
                            uv[:, half * 512:(half + 1) * 512],
                            start=True,
                            stop=True,
                        )
                        if half == 0:
                            nc.vector.tensor_copy(
                                scrow[0:1, 0:512], sc_ps
                            )
                        else:
                            nc.scalar.copy(scrow[0:1, 512:1024], sc_ps)
                    scd = dram_pool.tile([1, BTG], F32, tag="scd")
                    nc.sync.dma_start(out=scd, in_=scrow)
                    nc.sync.dma_start(
                        out=stages[c][gl * 8:gl * 8 + 8, :],
                        in_=scd.rearrange("a (r q) -> (a r) q", q=PH),
                    )

                    # ---- half-boundary work: softmax for c=0 is issued
                    # after group 8's mm1 so the PE transpose never waits
                    # on the stage scatter DMAs ----
                    if g == GPH:
                        softmax_half(0)
                    if g >= GPH + 1:
                        # spread the 64 c=0 chunks over groups 9..15
                        issue_step4(9 if g > GPH + 1 else 10)

                softmax_half(1)
                issue_step4(2 * B - step4_done)

                nc.vector.tensor_copy(s_sb, s_ps)
                nc.sync.dma_start(out=out, in_=s_sb)

    nc.compile()
    return nc


_prog_cache = {}


def _get_program(b_dim=B):
    if "p" not in _prog_cache:
        _prog_cache["p"] = build_program()
    return _prog_cache["p"]


def prep_core_inputs(shard_f32, w, bias, v):
    """Pack one core's [B, T_LOC, D] f32 shard + weights into device
    layouts. Row order: r = c*8192 + b*128 + t_lo."""
    h16 = shard_f32.astype(np.float16)          # [64, 256, 256]
    # [b, c, t_lo, d]
    hr = h16.reshape(B, 2, PH, D)
    # xn16[c, gl, p=t_lo, i=b%8, d]
    xn16 = np.ascontiguousarray(
        hr.reshape(8, 8, 2, PH, D).transpose(2, 0, 3, 1, 4)
    ).reshape(2, GPH, PH, 8, D)
    # xt16[g, p, m, q] = X[row=(c, gl*8+i, q), dc*128+p]
    #   hr -> [b, c, q, dc, p] -> [c, gl, i, q, dc, p] -> [c, gl, p, i, dc, q]
    hx = hr.reshape(8, 8, 2, PH, 2, PH).transpose(2, 0, 5, 1, 4, 3)
    xt16 = np.ascontiguousarray(hx).reshape(NG, PH, 16, PH)
    # wt16[p, dc, ec, e'] = W[ec*128 + e', dc*128 + p]
    w16 = w.astype(np.float16)
    wt16 = np.ascontiguousarray(
        w16.reshape(2, PH, 2, PH).transpose(3, 2, 0, 1)
    )
    bias2 = np.ascontiguousarray(bias.reshape(2, PH).T).astype(np.float32)
    v2 = np.ascontiguousarray(v.reshape(2, PH).T).astype(np.float32)
    return {
        "xn16": xn16,
        "xt16": xt16,
        "wt16": wt16,
        "bias2": bias2,
        "v2": v2,
    }


def kernel(hidden_states, W_attention, bias_attention, attention_vector):
    from concourse.bass_utils import run_bass_kernel_spmd

    hs = np.asarray(hidden_states, dtype=np.float32)
    w = np.asarray(W_attention, dtype=np.float32)
    bias = np.asarray(bias_attention, dtype=np.float32)
    v = np.asarray(attention_vector, dtype=np.float32)

    nc = _get_program()

    in_maps = []
    for core in range(NCORES):
        shard = np.ascontiguousarray(
            hs[:, core * T_LOC:(core + 1) * T_LOC, :]
        )
        in_maps.append(prep_core_inputs(shard, w, bias, v))

    res = run_bass_kernel_spmd(nc, in_maps, list(range(NCORES)))
    s = np.zeros((B, D), dtype=np.float32)
    for r in res.results:
        s += r["out"]
    return s
